# revision 1
# baseline (speedup 1.0000x reference)
"""Trainium2 Bass kernel for nn_DiffusionActionHead (B=8, S=2048, D=4096).

Strategy (8 NeuronCores):
  - Data-parallel over batch for everything touching llm_output (32 MiB/core).
  - Tensor-parallel weight reads: core i reads column-slice i of wq/wk/wv,
    row-slice i of wo, column/row slice i of mlp_w1/mlp_w2 (~96 MiB of
    weights split 8 ways), tiny diffusion tail replicated.
  - MAP-head attention with q_len=1 is collapsed algebraically:
        scores[s,h] = llm[s,:] . U[:,h],   U[:,h] = wk[:,hb] @ q_h / sqrt(DH)
        pooled[h,:] = softmax(scores)[h,:] @ llm
        ctx[hb]     = pooled[h,:] @ wv[:,hb] + bv[hb]
    (bk shifts scores by a per-head constant -> cancels in softmax.)
  - 4 small collectives: AllGather(U cols), AllToAll(pooled, head<->batch),
    AllReduce(attn_out partial), AllReduce(mlp partial).
  - Large matmuls run in fp16 (1 cyc/row on PE, half the HBM bytes); all
    accumulation, softmax, layernorms and residuals stay fp32.
  - Biases are folded into the PSUM accumulations via a ones-row matmul;
    additive biases of AllReduce'd partials are pre-divided by 8 on host.
  - Activations are kept feature-on-partition ("transposed") so every big
    matmul streams its weight slice in natural DRAM layout as the moving
    operand; llm itself is passed in both layouts (llmT host-transposed).
  - Two HWDGE queues: sync carries the llm streams, scalar carries the
    weight streams, so a stalled stream never head-of-line-blocks the other.
"""

import numpy as np
import sys

if "/opt/trn_rl_repo" not in sys.path:
    sys.path.insert(0, "/opt/trn_rl_repo")

import concourse.bass as bass
import concourse.tile as tile
from concourse import bacc, mybir
from concourse.masks import make_identity
from concourse.bass_utils import run_bass_kernel_spmd

F32 = mybir.dt.float32
F16 = mybir.dt.float16
AF = mybir.ActivationFunctionType
ALU = mybir.AluOpType

B, S, D = 8, 2048, 4096
H, AD, TD, HID, NBLK = 8, 7, 32, 256, 3
DH = D // H            # 512
NC = 8                 # cores
P = 128
SC = S // P            # 16 S-chunks
DC = D // P            # 32 D-chunks
HD2 = D // 2           # 2048 (half width -> 4-bank PSUM tiles)
F1S = 4 * D // NC      # 2048 per-core hidden cols of mlp_w1
HC = HID // P          # 2
RSQRT_DH = 1.0 / float(np.sqrt(DH))
TWO_PI = 2.0 * float(np.pi)


def _bcast(src_ap, nparts):
    """Partition-broadcast a (1, N) DRAM AP to (nparts, N)."""
    ap = src_ap
    assert ap.shape[0] == 1, ap.shape
    return bass.AP(tensor=ap.tensor, offset=ap.offset,
                   ap=[[0, nparts]] + [list(x) for x in ap.ap[1:]])


def build_program():
    nc = bacc.Bacc("TRN2", target_bir_lowering=False, debug=False,
                   num_devices=NC)
    t = {}

    def din(name, shape, dtype=F32):
        t[name] = nc.dram_tensor(name, shape, dtype, kind="ExternalInput")

    din("llm", [S, D], F16); din("llmT", [D, S], F16)
    din("wq_s", [D, DH], F16); din("bq_s", [1, DH])
    din("wkT_s", [DH, D], F16)
    din("wv_s", [D, DH], F16); din("bv16", [1, DH], F16)
    din("wo_s", [DH, D], F16); din("bo16", [1, D], F16)        # bo/8
    din("ln_g_r", [P, DC]); din("ln_b_r", [P, DC])
    din("w1_s", [D, F1S], F16); din("b116", [1, F1S], F16)
    din("w2_s", [F1S, D], F16); din("b216", [1, D], F16)       # b2/8
    din("probe_r", [P, DC], F16)
    din("four_w2", [TD, 1]); din("phase2", [TD, 1])
    din("timeT", [1, B]); din("naT", [AD, B], F16)
    din("cond_w1", [TD, 2 * TD], F16); din("cond_b1c", [2 * TD, 1])
    din("cond_w2", [2 * TD, TD], F16); din("cond_b2c", [TD, 1])
    din("rin_cond", [TD, HID], F16); din("rin_pool", [D, HID], F16)
    din("rin_na", [AD, HID], F16); din("rb16", [1, HID], F16)
    din("blk_g_r", [NBLK, P, HC]); din("blk_b_r", [NBLK, P, HC])
    din("blk_w1", [NBLK, HID, 4 * HID], F16)
    din("blk_b1_16", [NBLK, 4 * HID], F16)
    din("blk_w2", [NBLK, 4 * HID, HID], F16)
    din("blk_b2_16", [NBLK, HID], F16)
    din("out_w", [HID, AD], F16); din("out_bc", [1, AD])
    t["res"] = nc.dram_tensor("res", [B, AD], F32, kind="ExternalOutput")

    # collective bounce buffers (internal DRAM; AG/AR outputs in Shared space)
    t["cc_u_in"] = nc.dram_tensor("cc_u_in", [1, D], F32)
    t["cc_u_out"] = nc.dram_tensor("cc_u_out", [NC, D], F32, addr_space="Shared")
    t["cc_pool_in"] = nc.dram_tensor("cc_pool_in", [H, D], F32)
    t["cc_pool_out"] = nc.dram_tensor("cc_pool_out", [B, D], F32)
    t["cc_attn_in"] = nc.dram_tensor("cc_attn_in", [B, D], F32)
    t["cc_attn_out"] = nc.dram_tensor("cc_attn_out", [B, D], F32,
                                      addr_space="Shared")
    t["cc_mlp_in"] = nc.dram_tensor("cc_mlp_in", [B, D], F32)
    t["cc_mlp_out"] = nc.dram_tensor("cc_mlp_out", [B, D], F32,
                                     addr_space="Shared")

    with tile.TileContext(nc) as tc:
        import contextlib
        with contextlib.ExitStack() as ctx:
            _build(nc, tc, t, ctx)
    nc.finalize()
    return nc


def _build(nc, tc, t, ctx):
    GROUPS = [list(range(NC))]

    singles = ctx.enter_context(tc.tile_pool(name="singles", bufs=1))
    llm_pool = ctx.enter_context(tc.tile_pool(name="llm_pool", bufs=6))
    llmT_pool = ctx.enter_context(tc.tile_pool(name="llmT_pool", bufs=8))
    wst = ctx.enter_context(tc.tile_pool(name="wst", bufs=8))
    nat16 = ctx.enter_context(tc.tile_pool(name="nat16", bufs=2))
    nat8 = ctx.enter_context(tc.tile_pool(name="nat8", bufs=2))
    psA = ctx.enter_context(tc.tile_pool(name="psA", bufs=1, space="PSUM"))
    psB = ctx.enter_context(tc.tile_pool(name="psB", bufs=2, space="PSUM"))
    psC = ctx.enter_context(tc.tile_pool(name="psC", bufs=2, space="PSUM"))

    ident = singles.tile([P, P], F32)
    make_identity(nc, ident)
    eps_sb = singles.tile([P, 1], F32)
    nc.vector.memset(eps_sb[:], 1e-5)
    ones8 = singles.tile([1, 8], F16)
    nc.vector.memset(ones8[:], 1.0)

    def evict(dst, src):
        nc.vector.tensor_copy(out=dst, in_=src)

    def t_nat_to_T(src_nat, dst_T, nchunks, npart, uid):
        """(npart, nchunks*128) sbuf -> (128, nchunks, npart) sbuf via PE."""
        for c in range(nchunks):
            ps = psB.tile([P, 8], F32, tag="tp8", name=f"tp_{uid}_{c}")
            nc.tensor.transpose(ps[:, :npart], src_nat[:, c * P:(c + 1) * P],
                                ident[:npart, :npart])
            evict(dst_T[:, c, :], ps[:, :npart])

    def bias_mm(ps, bias_row, n_total, stop=True):
        """Add a (1, n_total) f16 bias row into psum (8, n_total) via ones-row
        matmuls, 512 cols per matmul (moving-dim limit)."""
        nch = (n_total + 511) // 512
        for n in range(nch):
            w = min(512, n_total - n * 512)
            nc.tensor.matmul(ps[:, n * 512:n * 512 + w], ones8[:, :B],
                             bias_row[:, n * 512:n * 512 + w],
                             start=False, stop=(stop and n == nch - 1))

    def layernorm_nat(x_nat, npart, n, y_nat, uid):
        """y = (x - mean) / sqrt(var + eps) over the free dim of (npart, n)."""
        nsub = max(1, n // 512)
        st = nat8.tile([npart, nsub, nc.vector.BN_STATS_DIM], F32, tag="lnst",
                       name=f"lnst_{uid}")
        xg = x_nat.rearrange("p (a b) -> p a b", a=nsub)
        for g in range(nsub):
            nc.vector.bn_stats(out=st[:, g, :], in_=xg[:, g, :])
        mv = nat8.tile([npart, nc.vector.BN_AGGR_DIM], F32, tag="lnmv",
                       name=f"lnmv_{uid}")
        nc.vector.bn_aggr(out=mv[:], in_=st[:])
        std = nat8.tile([npart, 1], F32, tag="lnsd", name=f"lnsd_{uid}")
        nc.scalar.activation(out=std[:], in_=mv[:, 1:2], func=AF.Sqrt,
                             bias=eps_sb[:npart, :])
        nc.vector.reciprocal(out=std[:], in_=std[:])
        nc.vector.tensor_scalar(out=y_nat, in0=x_nat, scalar1=mv[:, 0:1],
                                scalar2=std[:], op0=ALU.subtract, op1=ALU.mult)

    # =======================================================================
    # STEP 0: small constants, bias rows, tail weights — all prefetched
    # early on idle queues so the tail phase never waits on them.
    # =======================================================================
    probe_sb = singles.tile([P, DC], F16)
    nc.sync.dma_start(out=probe_sb[:], in_=t["probe_r"][:])
    bq_sb = singles.tile([1, DH], F32)
    nc.sync.dma_start(out=bq_sb[:], in_=t["bq_s"][:])
    bv_sb = singles.tile([1, DH], F16)
    nc.gpsimd.dma_start(out=bv_sb[:], in_=t["bv16"][:])
    bo_sb = singles.tile([1, D], F16)
    nc.gpsimd.dma_start(out=bo_sb[:], in_=t["bo16"][:])
    b1_sb = singles.tile([1, F1S], F16)
    nc.gpsimd.dma_start(out=b1_sb[:], in_=t["b116"][:])
    b2_sb = singles.tile([1, D], F16)
    nc.gpsimd.dma_start(out=b2_sb[:], in_=t["b216"][:])
    rb_sb = singles.tile([1, HID], F16)
    nc.gpsimd.dma_start(out=rb_sb[:], in_=t["rb16"][:])
    bb1_sb = singles.tile([1, NBLK, 4 * HID], F16)
    nc.gpsimd.dma_start(out=bb1_sb[:], in_=t["blk_b1_16"][:].rearrange("n f -> (n f)")[None, :])
    bb2_sb = singles.tile([1, NBLK, HID], F16)
    nc.gpsimd.dma_start(out=bb2_sb[:], in_=t["blk_b2_16"][:].rearrange("n f -> (n f)")[None, :])
    lng_sb = singles.tile([P, DC], F32)
    nc.sync.dma_start(out=lng_sb[:], in_=t["ln_g_r"][:])
    lnb_sb = singles.tile([P, DC], F32)
    nc.sync.dma_start(out=lnb_sb[:], in_=t["ln_b_r"][:])
    bgr_sb = singles.tile([P, NBLK, HC], F32)
    nc.sync.dma_start(out=bgr_sb[:],
                      in_=t["blk_g_r"][:].rearrange("n p c -> p n c"))
    bbr_sb = singles.tile([P, NBLK, HC], F32)
    nc.sync.dma_start(out=bbr_sb[:],
                      in_=t["blk_b_r"][:].rearrange("n p c -> p n c"))
    rc_sb = singles.tile([TD, HID], F16)
    nc.scalar.dma_start(out=rc_sb[:], in_=t["rin_cond"][:])
    rna_sb = singles.tile([AD, HID], F16)
    nc.scalar.dma_start(out=rna_sb[:], in_=t["rin_na"][:])
    naT_sb = singles.tile([AD, B], F16)
    nc.sync.dma_start(out=naT_sb[:], in_=t["naT"][:])
    ow_sb = singles.tile([P, HC, AD], F16)
    nc.sync.dma_start(out=ow_sb[:],
                      in_=t["out_w"][:].rearrange("(c p) a -> p c a", p=P))
    ob_bc = singles.tile([B, AD], F32)
    nc.gpsimd.dma_start(out=ob_bc[:], in_=_bcast(t["out_bc"][:], B))

    # =======================================================================
    # STEP 1: q = (probe @ wq_s + bq) / sqrt(DH)    -> (1, 512) natural
    # wq is streamed in 8 half-MiB DMAs (4 k-chunks each) on the scalar ring.
    # =======================================================================
    q_nat = singles.tile([1, DH], F32)
    ps_q = psC.tile([1, DH], F32, tag="vec", name="ps_q")
    wq_r = t["wq_s"].rearrange("(c p) n -> p c n", p=P)
    for g in range(8):
        wt = wst.tile([P, 4, DH], F16, tag="wst", name=f"wq_g{g}")
        nc.scalar.dma_start(out=wt[:], in_=wq_r[:, 4 * g:4 * g + 4, :])
        for j in range(4):
            k = 4 * g + j
            nc.tensor.matmul(ps_q[:], probe_sb[:, k:k + 1], wt[:, j, :],
                             start=(k == 0), stop=(k == DC - 1))
    nc.vector.tensor_add(out=q_nat[:], in0=ps_q[:], in1=bq_sb[:])
    nc.vector.tensor_scalar_mul(out=q_nat[:], in0=q_nat[:], scalar1=RSQRT_DH)

    qT = singles.tile([P, DH // P], F16)  # (128, 4)
    for c in range(DH // P):
        ps = psB.tile([P, 8], F32, tag="tp8", name=f"tp_q_{c}")
        nc.tensor.transpose(ps[:, :1], q_nat[:, c * P:(c + 1) * P], ident[:1, :1])
        evict(qT[:, c:c + 1], ps[:, :1])

    # =======================================================================
    # STEP 2: U column of this core's head: U = wkT_s.T @ q~  -> (1, 4096)
    #         AllGather -> cc_u_out (8, 4096) = U.T with one row per head
    # =======================================================================
    u_nat = nat16.tile([1, D], F32, tag="nat16", name="u_nat")
    for nhalf in range(2):
        wk_tiles = []
        for k in range(DH // P):
            wt = wst.tile([P, HD2], F16, tag="wst", name=f"wk_t{nhalf}_{k}")
            nc.scalar.dma_start(
                out=wt[:],
                in_=t["wkT_s"][k * P:(k + 1) * P, nhalf * HD2:(nhalf + 1) * HD2])
            wk_tiles.append(wt)
        for ncol in range(4):
            n0 = nhalf * 4 + ncol
            ps_u = psC.tile([1, DH], F32, tag="vec", name=f"ps_u_{n0}")
            for k in range(DH // P):
                nc.tensor.matmul(
                    ps_u[:], qT[:, k:k + 1],
                    wk_tiles[k][:, ncol * DH:(ncol + 1) * DH],
                    start=(k == 0), stop=(k == DH // P - 1))
            evict(u_nat[:, n0 * DH:(n0 + 1) * DH], ps_u[:])

    nc.gpsimd.dma_start(out=t["cc_u_in"][:], in_=u_nat[:])
    nc.gpsimd.collective_compute(
        "AllGather", ALU.bypass, replica_groups=GROUPS,
        ins=[t["cc_u_in"][:].opt()], outs=[t["cc_u_out"][:].opt()])

    # ---- cond path (fourier + tiny mlp) — independent of everything above,
    # computed here so it is off the critical path of the tail.
    fw_sb = singles.tile([TD, 1], F32)
    nc.sync.dma_start(out=fw_sb[:], in_=t["four_w2"][:])
    ph_sb = singles.tile([TD, 1], F32)
    nc.sync.dma_start(out=ph_sb[:], in_=t["phase2"][:])
    tb32 = singles.tile([TD, B], F32)
    nc.gpsimd.dma_start(out=tb32[:], in_=_bcast(t["timeT"][:], TD))
    fu = singles.tile([TD, B], F32)
    nc.vector.tensor_scalar_mul(out=fu[:], in0=tb32[:], scalar1=fw_sb[:])
    # exact range reduction: sin/cos have period 1 in fu, so subtract the
    # integer part via an f32->i32->f32 round-trip (|fu| < ~64 here).
    fi = singles.tile([TD, B], mybir.dt.int32)
    nc.vector.tensor_copy(out=fi[:], in_=fu[:])
    fif = singles.tile([TD, B], F32)
    nc.vector.tensor_copy(out=fif[:], in_=fi[:])
    nc.vector.tensor_sub(out=fu[:], in0=fu[:], in1=fif[:])
    ffT = singles.tile([TD, B], F16)
    nc.scalar.activation(out=ffT[:], in_=fu[:], func=AF.Sin,
                         scale=TWO_PI, bias=ph_sb[:])
    cw1_sb = singles.tile([TD, 2 * TD], F16)
    nc.scalar.dma_start(out=cw1_sb[:], in_=t["cond_w1"][:])
    cb1_sb = singles.tile([2 * TD, 1], F32)
    nc.sync.dma_start(out=cb1_sb[:], in_=t["cond_b1c"][:])
    cw2_sb = singles.tile([2 * TD, TD], F16)
    nc.scalar.dma_start(out=cw2_sb[:], in_=t["cond_w2"][:])
    cb2_sb = singles.tile([TD, 1], F32)
    nc.sync.dma_start(out=cb2_sb[:], in_=t["cond_b2c"][:])
    ps_c1 = psB.tile([P, 8], F32, tag="tp8", name="ps_c1")
    nc.tensor.matmul(ps_c1[:2 * TD, :B], cw1_sb[:], ffT[:], start=True, stop=True)
    c1 = singles.tile([2 * TD, B], F16)
    nc.scalar.activation(out=c1[:], in_=ps_c1[:2 * TD, :B], func=AF.Silu,
                         bias=cb1_sb[:])
    ps_c2 = psB.tile([P, 8], F32, tag="tp8", name="ps_c2")
    nc.tensor.matmul(ps_c2[:TD, :B], cw2_sb[:], c1[:], start=True, stop=True)
    condT = singles.tile([TD, B], F16)
    nc.scalar.activation(out=condT[:], in_=ps_c2[:TD, :B], func=AF.Identity,
                         bias=cb2_sb[:])

    # ---- read back U.T (8, 4096), transpose to (128, 32, 8), cast to f16
    uh_nat = nat16.tile([H, D], F32, tag="nat16", name="uh_nat")
    nc.sync.dma_start(out=uh_nat[:], in_=t["cc_u_out"][:])
    u_f16 = singles.tile([P, DC, H], F16)
    for c in range(DC):
        ps = psB.tile([P, 8], F32, tag="tp8", name=f"tp_u_{c}")
        nc.tensor.transpose(ps[:, :H], uh_nat[:, c * P:(c + 1) * P],
                            ident[:H, :H])
        evict(u_f16[:, c, :], ps[:, :H])

    # =======================================================================
    # STEP 3: scoresT (8, 2048) = U.T @ llmT  (fp16 inputs, fp32 accum)
    # =======================================================================
    ps_sc = psA.tile([H, S], F32, tag="big", name="ps_sc")
    for k in range(DC):
        lt = llmT_pool.tile([P, S], F16, tag="llmT", name=f"llmT_t{k}")
        nc.sync.dma_start(out=lt[:], in_=t["llmT"][k * P:(k + 1) * P, :])
        for n in range(S // 512):
            nc.tensor.matmul(ps_sc[:, n * 512:(n + 1) * 512],
                             u_f16[:, k, :], lt[:, n * 512:(n + 1) * 512],
                             start=(k == 0), stop=(k == DC - 1))

    # =======================================================================
    # STEP 4: softmax over S. Max-subtraction is skipped deliberately:
    # softmax is shift-invariant and |scores| here is < ~1, so exp() is
    # perfectly conditioned; the result is mathematically identical.
    # =======================================================================
    p_nat = nat8.tile([H, S], F32, tag="nat8", name="p_nat")
    nc.scalar.activation(out=p_nat[:], in_=ps_sc[:], func=AF.Exp)
    den = singles.tile([H, 1], F32)
    nc.vector.reduce_sum(out=den[:], in_=p_nat[:], axis=mybir.AxisListType.X)
    nc.vector.reciprocal(out=den[:], in_=den[:])
    nc.vector.tensor_scalar_mul(out=p_nat[:], in0=p_nat[:], scalar1=den[:])
    pT = singles.tile([P, SC, H], F16)
    t_nat_to_T(p_nat, pT, SC, H, "p")

    # =======================================================================
    # STEP 5: pooled (8, 4096) = pT.T @ llm ; AllToAll (head <-> batch)
    # =======================================================================
    pooled_nat = nat16.tile([H, D], F32, tag="nat16", name="pooled_nat")
    for half in range(2):
        ps_p = psA.tile([H, HD2], F32, tag="big", name=f"ps_pool_{half}")
        for s in range(SC):
            lt = llm_pool.tile([P, HD2], F16, tag="llm", name=f"llm_t{half}_{s}")
            nc.sync.dma_start(
                out=lt[:],
                in_=t["llm"][s * P:(s + 1) * P, half * HD2:(half + 1) * HD2])
            for n in range(HD2 // 512):
                nc.tensor.matmul(ps_p[:, n * 512:(n + 1) * 512],
                                 pT[:, s, :], lt[:, n * 512:(n + 1) * 512],
                                 start=(s == 0), stop=(s == SC - 1))
        evict(pooled_nat[:, half * HD2:(half + 1) * HD2], ps_p[:])

    nc.gpsimd.dma_start(out=t["cc_pool_in"][:], in_=pooled_nat[:])
    nc.gpsimd.collective_compute(
        "AllToAll", ALU.bypass, replica_groups=GROUPS,
        ins=[t["cc_pool_in"][:].opt()], outs=[t["cc_pool_out"][:].opt()])

    # =======================================================================
    # STEP 6: ctx for this core's head, all batches: (8, 512) = poolh@wv + bv
    # =======================================================================
    poolh_nat = nat16.tile([B, D], F32, tag="nat16", name="poolh_nat")
    nc.sync.dma_start(out=poolh_nat[:], in_=t["cc_pool_out"][:])
    poolhT = singles.tile([P, DC, B], F16)
    t_nat_to_T(poolh_nat, poolhT, DC, B, "ph")

    ps_cx = psA.tile([B, DH], F32, tag="big", name="ps_cx")
    wv_r = t["wv_s"].rearrange("(c p) n -> p c n", p=P)
    for g in range(8):
        wt = wst.tile([P, 4, DH], F16, tag="wst", name=f"wv_g{g}")
        nc.scalar.dma_start(out=wt[:], in_=wv_r[:, 4 * g:4 * g + 4, :])
        for j in range(4):
            k = 4 * g + j
            nc.tensor.matmul(ps_cx[:], poolhT[:, k, :], wt[:, j, :],
                             start=(k == 0), stop=False)
    bias_mm(ps_cx, bv_sb, DH)
    ctx_nat = nat8.tile([B, DH], F32, tag="nat8", name="ctx_nat")
    evict(ctx_nat[:], ps_cx[:])
    ctxT = singles.tile([P, DH // P, B], F16)
    t_nat_to_T(ctx_nat, ctxT, DH // P, B, "cx")

    # =======================================================================
    # STEP 7: attn partial (8, 4096) = ctx @ wo_s + bo/8 ; AllReduce
    # =======================================================================
    attn_part = nat16.tile([B, D], F32, tag="nat16", name="attn_part")
    for half in range(2):
        ps_a = psA.tile([B, HD2], F32, tag="big", name=f"ps_attn_{half}")
        for k in range(DH // P):
            wt = wst.tile([P, HD2], F16, tag="wst", name=f"wo_t{half}_{k}")
            nc.scalar.dma_start(
                out=wt[:],
                in_=t["wo_s"][k * P:(k + 1) * P, half * HD2:(half + 1) * HD2])
            for n in range(HD2 // 512):
                nc.tensor.matmul(ps_a[:, n * 512:(n + 1) * 512],
                                 ctxT[:, k, :], wt[:, n * 512:(n + 1) * 512],
                                 start=(k == 0), stop=False)
        bias_mm(ps_a, bo_sb[:, half * HD2:(half + 1) * HD2], HD2)
        evict(attn_part[:, half * HD2:(half + 1) * HD2], ps_a[:])
    nc.gpsimd.dma_start(out=t["cc_attn_in"][:], in_=attn_part[:])
    nc.gpsimd.collective_compute(
        "AllReduce", ALU.add, replica_groups=GROUPS,
        ins=[t["cc_attn_in"][:].opt()], outs=[t["cc_attn_out"][:].opt()])

    # =======================================================================
    # STEP 8: y = LN(attn_out)*g+b ; mlp partial (+b1, gelu, @w2 + b2/8) ; AR
    # =======================================================================
    attn_nat = singles.tile([B, D], F32)  # persists (residual)
    nc.sync.dma_start(out=attn_nat[:], in_=t["cc_attn_out"][:])

    y_nat = nat16.tile([B, D], F32, tag="nat16", name="y_nat")
    layernorm_nat(attn_nat[:], B, D, y_nat[:], "ln0")
    yT = singles.tile([P, DC, B], F16)
    t_nat_to_T(y_nat, yT, DC, B, "y")
    # LN affine in T layout (gamma/beta become per-partition scalars)
    for c in range(DC):
        nc.vector.tensor_scalar(out=yT[:, c, :], in0=yT[:, c, :],
                                scalar1=lng_sb[:, c:c + 1],
                                scalar2=lnb_sb[:, c:c + 1],
                                op0=ALU.mult, op1=ALU.add)

    # mm1: h1 (8, 2048) = y @ w1_s + b1 ; exact gelu straight off PSUM
    ps_h1 = psA.tile([B, F1S], F32, tag="big", name="ps_h1")
    for k in range(DC):
        wt = wst.tile([P, F1S], F16, tag="wst", name=f"w1_t{k}")
        nc.scalar.dma_start(out=wt[:], in_=t["w1_s"][k * P:(k + 1) * P, :])
        for n in range(F1S // 512):
            nc.tensor.matmul(ps_h1[:, n * 512:(n + 1) * 512],
                             yT[:, k, :], wt[:, n * 512:(n + 1) * 512],
                             start=(k == 0), stop=False)
    bias_mm(ps_h1, b1_sb, F1S)
    g_nat = nat8.tile([B, F1S], F32, tag="nat8", name="g_nat")
    nc.scalar.activation(out=g_nat[:], in_=ps_h1[:], func=AF.Gelu)
    gT = singles.tile([P, F1S // P, B], F16)
    t_nat_to_T(g_nat, gT, F1S // P, B, "g")

    # mm2: h2 partial (8, 4096) = g @ w2_s + b2/8 ; AllReduce
    h2_nat = nat16.tile([B, D], F32, tag="nat16", name="h2_nat")
    for half in range(2):
        ps_h2 = psA.tile([B, HD2], F32, tag="big", name=f"ps_h2_{half}")
        for k in range(F1S // P):
            wt = wst.tile([P, HD2], F16, tag="wst", name=f"w2_t{half}_{k}")
            nc.scalar.dma_start(
                out=wt[:],
                in_=t["w2_s"][k * P:(k + 1) * P, half * HD2:(half + 1) * HD2])
            for n in range(HD2 // 512):
                nc.tensor.matmul(ps_h2[:, n * 512:(n + 1) * 512],
                                 gT[:, k, :], wt[:, n * 512:(n + 1) * 512],
                                 start=(k == 0), stop=False)
        bias_mm(ps_h2, b2_sb[:, half * HD2:(half + 1) * HD2], HD2)
        evict(h2_nat[:, half * HD2:(half + 1) * HD2], ps_h2[:])
    nc.gpsimd.dma_start(out=t["cc_mlp_in"][:], in_=h2_nat[:])
    nc.gpsimd.collective_compute(
        "AllReduce", ALU.add, replica_groups=GROUPS,
        ins=[t["cc_mlp_in"][:].opt()], outs=[t["cc_mlp_out"][:].opt()])

    # =======================================================================
    # STEP 9: x_pool = attn_out + h ; diffusion tail (replicated on all cores)
    # =======================================================================
    hug = nat16.tile([B, D], F32, tag="nat16", name="hug")
    nc.sync.dma_start(out=hug[:], in_=t["cc_mlp_out"][:])
    nc.vector.tensor_add(out=attn_nat[:], in0=attn_nat[:], in1=hug[:])
    xpT = singles.tile([P, DC, B], F16)
    t_nat_to_T(attn_nat, xpT, DC, B, "xp")

    # x0 (8, 256) = x_pool@rin_pool + cond@rin_cond + na@rin_na + rin_b
    ps_x0 = psA.tile([B, HID], F32, tag="big", name="ps_x0")
    for k in range(DC):
        wt = wst.tile([P, HID], F16, tag="wst", name=f"rp_t{k}")
        nc.scalar.dma_start(out=wt[:], in_=t["rin_pool"][k * P:(k + 1) * P, :])
        nc.tensor.matmul(ps_x0[:], xpT[:, k, :], wt[:], start=(k == 0),
                         stop=False)
    nc.tensor.matmul(ps_x0[:], condT[:], rc_sb[:], start=False, stop=False)
    nc.tensor.matmul(ps_x0[:], naT_sb[:], rna_sb[:], start=False, stop=False)
    bias_mm(ps_x0, rb_sb, HID)
    x_nat = singles.tile([B, HID], F32)
    evict(x_nat[:], ps_x0[:])

    # ---- 3 residual blocks ----
    for i in range(NBLK):
        xn = singles.tile([B, HID], F32, name=f"xn_{i}")
        layernorm_nat(x_nat[:], B, HID, xn[:], f"lnb{i}")
        xnT = singles.tile([P, HC, B], F16, name=f"xnT_{i}")
        t_nat_to_T(xn, xnT, HC, B, f"xn{i}")
        for c in range(HC):  # LN affine in T layout
            nc.vector.tensor_scalar(out=xnT[:, c, :], in0=xnT[:, c, :],
                                    scalar1=bgr_sb[:, i, c:c + 1],
                                    scalar2=bbr_sb[:, i, c:c + 1],
                                    op0=ALU.mult, op1=ALU.add)

        ps_bh = psA.tile([B, 4 * HID], F32, tag="big", name=f"ps_bh_{i}")
        for k in range(HC):
            wt = wst.tile([P, 4 * HID], F16, tag="wst", name=f"bw1_t{i}_{k}")
            nc.scalar.dma_start(out=wt[:], in_=t["blk_w1"][i, k * P:(k + 1) * P, :])
            for n in range(4 * HID // 512):
                nc.tensor.matmul(ps_bh[:, n * 512:(n + 1) * 512],
                                 xnT[:, k, :], wt[:, n * 512:(n + 1) * 512],
                                 start=(k == 0), stop=False)
        bias_mm(ps_bh, bb1_sb[:, i, :], 4 * HID)
        hb = nat8.tile([B, 4 * HID], F32, tag="nat8", name=f"hb_{i}")
        nc.scalar.activation(out=hb[:], in_=ps_bh[:], func=AF.Silu)
        hbT = singles.tile([P, 4 * HID // P, B], F16, name=f"hbT_{i}")
        t_nat_to_T(hb, hbT, 4 * HID // P, B, f"hb{i}")

        ps_bo = psA.tile([B, HID], F32, tag="big", name=f"ps_bo_{i}")
        for k in range(4 * HID // P):
            wt = wst.tile([P, HID], F16, tag="wst", name=f"bw2_t{i}_{k}")
            nc.scalar.dma_start(out=wt[:], in_=t["blk_w2"][i, k * P:(k + 1) * P, :])
            nc.tensor.matmul(ps_bo[:], hbT[:, k, :], wt[:],
                             start=(k == 0), stop=False)
        bias_mm(ps_bo, bb2_sb[:, i, :], HID)
        nc.vector.tensor_add(out=x_nat[:], in0=x_nat[:], in1=ps_bo[:])

    # ---- final: res (8, 7) = swish(x) @ out_w + out_b
    nc.scalar.activation(out=x_nat[:], in_=x_nat[:], func=AF.Silu)
    xsT = singles.tile([P, HC, B], F16)
    t_nat_to_T(x_nat, xsT, HC, B, "xs")
    ps_o = psB.tile([P, 8], F32, tag="tp8", name="ps_o")
    for k in range(HC):
        nc.tensor.matmul(ps_o[:B, :AD], xsT[:, k, :], ow_sb[:, k, :],
                         start=(k == 0), stop=(k == HC - 1))
    out_sb = singles.tile([B, AD], F32)
    nc.vector.tensor_add(out=out_sb[:], in0=ps_o[:B, :AD], in1=ob_bc[:])
    nc.sync.dma_start(out=t["res"][:], in_=out_sb[:])


_CACHED_NC = None


def _get_nc():
    global _CACHED_NC
    if _CACHED_NC is None:
        _CACHED_NC = build_program()
    return _CACHED_NC


def _prep_in_maps(inputs):
    f32 = np.float32
    f16 = np.float16
    llm_full = np.ascontiguousarray(np.asarray(inputs["llm_output"], dtype=f32))
    wq = np.asarray(inputs["wq"], f32); wk = np.asarray(inputs["wk"], f32)
    wv = np.asarray(inputs["wv"], f32); wo = np.asarray(inputs["wo"], f32)
    bq = np.asarray(inputs["bq"], f32); bv = np.asarray(inputs["bv"], f32)
    bo = np.asarray(inputs["bo"], f32)
    w1 = np.asarray(inputs["mlp_w1"], f32); b1 = np.asarray(inputs["mlp_b1"], f32)
    w2 = np.asarray(inputs["mlp_w2"], f32); b2 = np.asarray(inputs["mlp_b2"], f32)
    rin_w = np.asarray(inputs["rin_w"], f32)
    probe = np.asarray(inputs["probe"], f32).reshape(D)

    def r128(v):  # (n*128,) -> (128, n) partition-major
        return np.ascontiguousarray(v.reshape(-1, P).T)

    blk_g = np.asarray(inputs["blk_ln_g"], f32)
    blk_b = np.asarray(inputs["blk_ln_b"], f32)

    shared = {
        "bo16": (bo / NC).astype(f16).reshape(1, D),
        "ln_g_r": r128(np.asarray(inputs["ln_g"], f32)),
        "ln_b_r": r128(np.asarray(inputs["ln_b"], f32)),
        "b216": (b2 / NC).astype(f16).reshape(1, D),
        "probe_r": r128(probe).astype(f16),
        "four_w2": np.concatenate(
            [np.asarray(inputs["four_w"], f32).reshape(TD // 2, 1)] * 2),
        "phase2": np.concatenate(
            [np.full((TD // 2, 1), np.pi / 2, f32),
             np.zeros((TD // 2, 1), f32)]),
        "timeT": np.ascontiguousarray(np.asarray(inputs["time"], f32).T),
        "naT": np.ascontiguousarray(
            np.asarray(inputs["noisy_actions"], f32).T).astype(f16),
        "cond_w1": np.asarray(inputs["cond_w1"], f32).astype(f16),
        "cond_b1c": np.asarray(inputs["cond_b1"], f32).reshape(-1, 1),
        "cond_w2": np.asarray(inputs["cond_w2"], f32).astype(f16),
        "cond_b2c": np.asarray(inputs["cond_b2"], f32).reshape(-1, 1),
        "rin_cond": np.ascontiguousarray(rin_w[0:TD]).astype(f16),
        "rin_pool": np.ascontiguousarray(rin_w[TD:TD + D]).astype(f16),
        "rin_na": np.ascontiguousarray(rin_w[TD + D:]).astype(f16),
        "rb16": np.asarray(inputs["rin_b"], f32).astype(f16).reshape(1, HID),
        "blk_g_r": np.ascontiguousarray(
            blk_g.reshape(NBLK, HC, P).transpose(0, 2, 1)),
        "blk_b_r": np.ascontiguousarray(
            blk_b.reshape(NBLK, HC, P).transpose(0, 2, 1)),
        "blk_w1": np.asarray(inputs["blk_w1"], f32).astype(f16),
        "blk_b1_16": np.asarray(inputs["blk_b1"], f32).astype(f16),
        "blk_w2": np.asarray(inputs["blk_w2"], f32).astype(f16),
        "blk_b2_16": np.asarray(inputs["blk_b2"], f32).astype(f16),
        "out_w": np.asarray(inputs["out_w"], f32).astype(f16),
        "out_bc": np.asarray(inputs["out_b"], f32).reshape(1, AD),
    }

    in_maps = []
    for i in range(NC):
        hb = slice(i * DH, (i + 1) * DH)
        fb = slice(i * F1S, (i + 1) * F1S)
        m = dict(shared)
        m["llm"] = llm_full[i].astype(f16)
        m["llmT"] = np.ascontiguousarray(llm_full[i].T).astype(f16)
        m["wq_s"] = np.ascontiguousarray(wq[:, hb]).astype(f16)
        m["bq_s"] = np.ascontiguousarray(bq[hb]).reshape(1, DH)
        m["wkT_s"] = np.ascontiguousarray(wk[:, hb].T).astype(f16)
        m["wv_s"] = np.ascontiguousarray(wv[:, hb]).astype(f16)
        m["bv16"] = np.ascontiguousarray(bv[hb]).astype(f16).reshape(1, DH)
        m["wo_s"] = np.ascontiguousarray(wo[hb, :]).astype(f16)
        m["w1_s"] = np.ascontiguousarray(w1[:, fb]).astype(f16)
        m["b116"] = np.ascontiguousarray(b1[fb]).astype(f16).reshape(1, F1S)
        m["w2_s"] = np.ascontiguousarray(w2[fb, :]).astype(f16)
        in_maps.append(m)
    return in_maps


def kernel(**inputs):
    nc = _get_nc()
    in_maps = _prep_in_maps(inputs)
    r = run_bass_kernel_spmd(nc, in_maps, core_ids=list(range(NC)))
    return np.ascontiguousarray(r.results[0]["res"]).astype(np.float32)


def run_traced(**inputs):
    """Like kernel() but with NTFF tracing; returns (output, results)."""
    nc = _get_nc()
    in_maps = _prep_in_maps(inputs)
    r = run_bass_kernel_spmd(nc, in_maps, core_ids=list(range(NC)), trace=True)
    return np.ascontiguousarray(r.results[0]["res"]).astype(np.float32), r



# revision 15
# speedup vs baseline: 1.1422x; 1.1422x over previous
"""Trainium2 Bass kernel for nn_DiffusionActionHead (B=8, S=2048, D=4096).

Strategy (8 NeuronCores), v2:
  - MAP-head probe projection is folded on HOST: U = wk_h^T (probe wq_h + bq_h)
    / sqrt(DH) is data-independent, so scores = llm @ U directly. This removes
    the wq/wk streams, the on-device q/U matmuls and the U AllGather entirely.
  - Data-parallel over batch for scores/softmax/pooled (each core owns one
    batch row of llm); AllToAll converts pooled to head-parallel for the
    wv/wo stage (each core reads only its head slice); AllReduce after wo and
    after the TP-sharded MLP (w1 col-shard, w2 row-shard). Tail replicated.
  - llmT is streamed in fp8 e3m4 (scores are softmax-shift tolerant; host
    scales llm by 2 into e3m4 range and stores U/2). llm natural stays f16
    for the pooled pass; all weights stay f16 (fp8 on weights was measured
    to breach the 2e-2 gate).
  - All collective payloads are f16. Collective staging writes + readbacks
    ride the gpsimd SWDGE queue so a semaphore-waiting DMA never head-of-line
    blocks a weight prefetch on the HWDGE rings.
  - Queue plan: sync = llmT8, llm16, w2, res; scalar = U/wv/wo/w1/tail;
    gpsimd = small constants + collective staging. Deep per-stream tile
    pools so every stream prefetches during earlier phases.
  - x0 = cond@rin_c + na@rin_na + rin_b accumulates into its PSUM bank at
    t~0 (start=True); the xp@rin_pool chunks land into the same bank after
    the last AllReduce.
  - All transposes are PE transposes with f16 in/out (PSUM f16 transpose
    path); LN affine and softmax denominators fold into the evict copies.
"""

import numpy as np
import sys

if "/opt/trn_rl_repo" not in sys.path:
    sys.path.insert(0, "/opt/trn_rl_repo")

import ml_dtypes
import concourse.bass as bass
import concourse.tile as tile
from concourse import bacc, mybir
from concourse.masks import make_identity
from concourse.bass_utils import run_bass_kernel_spmd

F32 = mybir.dt.float32
F16 = mybir.dt.float16
F8 = mybir.dt.float8e3
AF = mybir.ActivationFunctionType
ALU = mybir.AluOpType

B, S, D = 8, 2048, 4096
H, AD, TD, HID, NBLK = 8, 7, 32, 256, 3
DH = D // H            # 512
NC = 8                 # cores
P = 128
SC = S // P            # 16 S-chunks
DC = D // P            # 32 D-chunks
HD2 = D // 2           # 2048
F1S = 4 * D // NC      # 2048 per-core hidden cols of mlp_w1
HC = HID // P          # 2
LLM_SCALE = 2.0        # llm * 2 fits e3m4 comfortably; U stored as U/2
TWO_PI = 2.0 * float(np.pi)


def _bcast(src_ap, nparts):
    """Partition-broadcast a (1, N) DRAM AP to (nparts, N)."""
    ap = src_ap
    assert ap.shape[0] == 1, ap.shape
    return bass.AP(tensor=ap.tensor, offset=ap.offset,
                   ap=[[0, nparts]] + [list(x) for x in ap.ap[1:]])


def build_program():
    nc = bacc.Bacc("TRN2", target_bir_lowering=False, debug=False,
                   num_devices=NC)
    t = {}

    def din(name, shape, dtype=F32):
        t[name] = nc.dram_tensor(name, shape, dtype, kind="ExternalInput")

    din("llmT8", [D, S], F8)
    din("llm16", [S, D], F16)
    din("U16r", [P, DC, H], F16)
    din("wv_s", [D, DH], F16); din("bv16", [1, DH], F16)
    din("wo_s", [DH, D], F16); din("bo16", [1, D], F16)        # bo/8
    din("ln_g_r", [P, DC]); din("ln_b_r", [P, DC])
    din("w1_s", [D, F1S], F16); din("b116", [1, F1S], F16)
    din("w2_s", [F1S, D], F16); din("b216", [1, D], F16)       # b2/8
    din("four_w2", [TD, 1]); din("phase2", [TD, 1])
    din("timeT", [1, B]); din("naT", [AD, B], F16)
    din("cond_w1", [TD, 2 * TD], F16); din("cond_b1c", [2 * TD, 1])
    din("cond_w2", [2 * TD, TD], F16); din("cond_b2c", [TD, 1])
    din("rin_cond", [TD, HID], F16); din("rinp", [P, DC, HID], F16)
    din("rin_na", [AD, HID], F16); din("rb16", [1, HID], F16)
    din("blk_g_r", [NBLK, P, HC]); din("blk_b_r", [NBLK, P, HC])
    din("blkw1p", [P, NBLK * HC, 4 * HID], F16)
    din("blk_b1_16", [NBLK, 4 * HID], F16)
    din("blkw2p", [P, NBLK, 8 * HID], F16)
    din("blk_b2_16", [NBLK, HID], F16)
    din("out_w", [P, HC, AD], F16); din("out_bc", [1, AD])
    t["res"] = nc.dram_tensor("res", [B, AD], F32, kind="ExternalOutput")

    # collective bounce buffers (f16 payloads)
    t["cc_pool_in"] = nc.dram_tensor("cc_pool_in", [H, D], F16)
    t["cc_pool_out"] = nc.dram_tensor("cc_pool_out", [B, D], F16)
    t["cc_attn_in"] = nc.dram_tensor("cc_attn_in", [B, D], F16)
    t["cc_attn_out"] = nc.dram_tensor("cc_attn_out", [B, D], F16,
                                      addr_space="Shared")
    t["cc_mlp_in"] = nc.dram_tensor("cc_mlp_in", [B, D], F16)
    t["cc_mlp_out"] = nc.dram_tensor("cc_mlp_out", [B, D], F16,
                                     addr_space="Shared")

    with tile.TileContext(nc) as tc:
        import contextlib
        with contextlib.ExitStack() as ctx:
            _build(nc, tc, t, ctx)
    nc.finalize()
    return nc


def _build(nc, tc, t, ctx):
    GROUPS = [list(range(NC))]

    singles = ctx.enter_context(tc.tile_pool(name="singles", bufs=1))
    l8pool = ctx.enter_context(tc.tile_pool(name="l8pool", bufs=3))
    llm_pool = ctx.enter_context(tc.tile_pool(name="llm_pool", bufs=3))
    wvo_pool = ctx.enter_context(tc.tile_pool(name="wvo_pool", bufs=8))
    w1_pool = ctx.enter_context(tc.tile_pool(name="w1_pool", bufs=6))
    w2_pool = ctx.enter_context(tc.tile_pool(name="w2_pool", bufs=6))
    natD = ctx.enter_context(tc.tile_pool(name="natD", bufs=3))
    nat8 = ctx.enter_context(tc.tile_pool(name="nat8", bufs=2))
    psA = ctx.enter_context(tc.tile_pool(name="psA", bufs=1, space="PSUM"))
    psB = ctx.enter_context(tc.tile_pool(name="psB", bufs=2, space="PSUM"))
    psC = ctx.enter_context(tc.tile_pool(name="psC", bufs=1, space="PSUM"))

    ident16 = singles.tile([P, P], F16)
    make_identity(nc, ident16)
    eps_sb = singles.tile([P, 1], F32)
    nc.vector.memset(eps_sb[:], 1e-5)
    ones8 = singles.tile([1, 8], F16)
    nc.vector.memset(ones8[:], 1.0)

    def evict(dst, src):
        nc.vector.tensor_copy(out=dst, in_=src)

    def t_T16(src, dst, nchunks, npart, uid, g=None, b=None):
        """(npart, nchunks*128) f16 sbuf -> (128, nchunks, npart) f16 sbuf
        via PE f16 transposes; optional per-chunk affine (g, b are (P, nchunks)
        f32 tiles applied per-partition on the transposed data)."""
        for c in range(nchunks):
            ps = psB.tile([P, 8], F16, tag="tp16", name=f"tp_{uid}_{c}")
            nc.tensor.transpose(ps[:, :npart], src[:, c * P:(c + 1) * P],
                                ident16[:npart, :npart])
            if g is not None:
                nc.vector.tensor_scalar(out=dst[:, c, :], in0=ps[:, :npart],
                                        scalar1=g[:, c:c + 1],
                                        scalar2=b[:, c:c + 1],
                                        op0=ALU.mult, op1=ALU.add)
            else:
                nc.vector.tensor_copy(out=dst[:, c, :], in_=ps[:, :npart])

    def bias_mm(ps, bias_row, n_total, stop=True):
        """Add a (1, n_total) f16 bias row into psum (8, n_total) via ones-row
        matmuls, 512 cols per matmul."""
        nch = (n_total + 511) // 512
        for n in range(nch):
            w = min(512, n_total - n * 512)
            nc.tensor.matmul(ps[:, n * 512:n * 512 + w], ones8[:, :B],
                             bias_row[:, n * 512:n * 512 + w],
                             start=False, stop=(stop and n == nch - 1))

    def layernorm16(x_in, npart, n, y16, uid):
        """y16 = f16((x - mean) / sqrt(var + eps)) over free dim of (npart, n).
        Input may be f16 or f32."""
        nsub = max(1, n // 512)
        st = nat8.tile([npart, nsub, nc.vector.BN_STATS_DIM], F32, tag="lnst",
                       name=f"lnst_{uid}")
        xg = x_in.rearrange("p (a b) -> p a b", a=nsub)
        for g in range(nsub):
            nc.vector.bn_stats(out=st[:, g, :], in_=xg[:, g, :])
        mv = nat8.tile([npart, nc.vector.BN_AGGR_DIM], F32, tag="lnmv",
                       name=f"lnmv_{uid}")
        nc.vector.bn_aggr(out=mv[:], in_=st[:])
        std = nat8.tile([npart, 1], F32, tag="lnsd", name=f"lnsd_{uid}")
        nc.scalar.activation(out=std[:], in_=mv[:, 1:2], func=AF.Sqrt,
                             bias=eps_sb[:npart, :])
        nc.vector.reciprocal(out=std[:], in_=std[:])
        nc.vector.tensor_scalar(out=y16, in0=x_in, scalar1=mv[:, 0:1],
                                scalar2=std[:], op0=ALU.subtract, op1=ALU.mult)

    # =======================================================================
    # STEP 0: small constants on gpsimd/scalar queues (prefetched early).
    # =======================================================================
    U16 = singles.tile([P, DC, H], F16)
    nc.scalar.dma_start(out=U16[:], in_=t["U16r"][:])
    cw1_sb = singles.tile([TD, 2 * TD], F16)
    nc.scalar.dma_start(out=cw1_sb[:], in_=t["cond_w1"][:])
    cw2_sb = singles.tile([2 * TD, TD], F16)
    nc.scalar.dma_start(out=cw2_sb[:], in_=t["cond_w2"][:])
    rc_sb = singles.tile([TD, HID], F16)
    nc.scalar.dma_start(out=rc_sb[:], in_=t["rin_cond"][:])
    rna_sb = singles.tile([AD, HID], F16)
    nc.scalar.dma_start(out=rna_sb[:], in_=t["rin_na"][:])

    bv_sb = singles.tile([1, DH], F16)
    nc.gpsimd.dma_start(out=bv_sb[:], in_=t["bv16"][:])
    bo_sb = singles.tile([1, D], F16)
    nc.gpsimd.dma_start(out=bo_sb[:], in_=t["bo16"][:])
    b1_sb = singles.tile([1, F1S], F16)
    nc.gpsimd.dma_start(out=b1_sb[:], in_=t["b116"][:])
    b2_sb = singles.tile([1, D], F16)
    nc.gpsimd.dma_start(out=b2_sb[:], in_=t["b216"][:])
    rb_sb = singles.tile([1, HID], F16)
    nc.gpsimd.dma_start(out=rb_sb[:], in_=t["rb16"][:])
    bb1_sb = singles.tile([1, NBLK, 4 * HID], F16)
    nc.gpsimd.dma_start(out=bb1_sb[:],
                        in_=t["blk_b1_16"][:].rearrange("n f -> (n f)")[None, :])
    bb2_sb = singles.tile([1, NBLK, HID], F16)
    nc.gpsimd.dma_start(out=bb2_sb[:],
                        in_=t["blk_b2_16"][:].rearrange("n f -> (n f)")[None, :])
    lng_sb = singles.tile([P, DC], F32)
    nc.gpsimd.dma_start(out=lng_sb[:], in_=t["ln_g_r"][:])
    lnb_sb = singles.tile([P, DC], F32)
    nc.gpsimd.dma_start(out=lnb_sb[:], in_=t["ln_b_r"][:])
    bgr_sb = singles.tile([P, NBLK, HC], F32)
    nc.gpsimd.dma_start(out=bgr_sb[:],
                        in_=t["blk_g_r"][:].rearrange("n p c -> p n c"))
    bbr_sb = singles.tile([P, NBLK, HC], F32)
    nc.gpsimd.dma_start(out=bbr_sb[:],
                        in_=t["blk_b_r"][:].rearrange("n p c -> p n c"))
    naT_sb = singles.tile([AD, B], F16)
    nc.gpsimd.dma_start(out=naT_sb[:], in_=t["naT"][:])
    ow_sb = singles.tile([P, HC, AD], F16)
    nc.gpsimd.dma_start(out=ow_sb[:], in_=t["out_w"][:])
    ob_bc = singles.tile([B, AD], F32)
    nc.gpsimd.dma_start(out=ob_bc[:], in_=_bcast(t["out_bc"][:], B))
    fw_sb = singles.tile([TD, 1], F32)
    nc.gpsimd.dma_start(out=fw_sb[:], in_=t["four_w2"][:])
    ph_sb = singles.tile([TD, 1], F32)
    nc.gpsimd.dma_start(out=ph_sb[:], in_=t["phase2"][:])
    cb1_sb = singles.tile([2 * TD, 1], F32)
    nc.gpsimd.dma_start(out=cb1_sb[:], in_=t["cond_b1c"][:])
    cb2_sb = singles.tile([TD, 1], F32)
    nc.gpsimd.dma_start(out=cb2_sb[:], in_=t["cond_b2c"][:])
    tb32 = singles.tile([TD, B], F32)
    nc.gpsimd.dma_start(out=tb32[:], in_=_bcast(t["timeT"][:], TD))

    # =======================================================================
    # STEP 1: cond path (fourier + tiny mlp) — independent of everything,
    # done first so condT exists before the early x0 accumulation.
    # =======================================================================
    fu = singles.tile([TD, B], F32)
    nc.vector.tensor_scalar_mul(out=fu[:], in0=tb32[:], scalar1=fw_sb[:])
    fi = singles.tile([TD, B], mybir.dt.int32)
    nc.vector.tensor_copy(out=fi[:], in_=fu[:])
    fif = singles.tile([TD, B], F32)
    nc.vector.tensor_copy(out=fif[:], in_=fi[:])
    nc.vector.tensor_sub(out=fu[:], in0=fu[:], in1=fif[:])
    ffT = singles.tile([TD, B], F16)
    nc.scalar.activation(out=ffT[:], in_=fu[:], func=AF.Sin,
                         scale=TWO_PI, bias=ph_sb[:])
    ps_c1 = psC.tile([2 * TD, B], F32, tag="mix", name="ps_c1")
    nc.tensor.matmul(ps_c1[:], cw1_sb[:], ffT[:], start=True, stop=True)
    c1 = singles.tile([2 * TD, B], F16)
    nc.scalar.activation(out=c1[:], in_=ps_c1[:], func=AF.Silu,
                         bias=cb1_sb[:])
    ps_c2 = psC.tile([TD, B], F32, tag="mix", name="ps_c2")
    nc.tensor.matmul(ps_c2[:], cw2_sb[:], c1[:], start=True, stop=True)
    condT = singles.tile([TD, B], F16)
    nc.scalar.activation(out=condT[:], in_=ps_c2[:], func=AF.Identity,
                         bias=cb2_sb[:])

    # x0 accumulation bank: cond + noisy_actions + bias land now, the
    # xp @ rin_pool chunks land after the mlp AllReduce.
    ps_x0 = psC.tile([B, HID], F32, tag="x0", name="ps_x0")
    nc.tensor.matmul(ps_x0[:], condT[:], rc_sb[:], start=True, stop=False)
    nc.tensor.matmul(ps_x0[:], naT_sb[:], rna_sb[:], start=False, stop=False)
    bias_mm(ps_x0, rb_sb, HID, stop=False)

    # =======================================================================
    # STEP 2: scoresT (8, 2048) = U.T @ llmT8  (f16 x fp8, fp32 accum)
    # =======================================================================
    ps_sc = psA.tile([H, S], F32, tag="big", name="ps_sc")
    for kt in range(DC // 2):
        lt = l8pool.tile([P, 2, S], F8, tag="l8", name=f"l8_{kt}")
        nc.sync.dma_start(
            out=lt[:],
            in_=t["llmT8"][kt * 256:(kt + 1) * 256, :].rearrange(
                "(a p) s -> p a s", a=2))
        for a in range(2):
            k = 2 * kt + a
            for n in range(S // 512):
                nc.tensor.matmul(ps_sc[:, n * 512:(n + 1) * 512],
                                 U16[:, k, :], lt[:, a, n * 512:(n + 1) * 512],
                                 start=(k == 0), stop=(k == DC - 1))

    # =======================================================================
    # STEP 3: softmax (shift-free: |scores| < ~0.2). The denominator is
    # folded into the pooled evict.
    # =======================================================================
    p16 = natD.tile([H, S], F16, tag="nat", name="p16")
    nc.scalar.activation(out=p16[:], in_=ps_sc[:], func=AF.Exp)
    den = singles.tile([H, 1], F32)
    nc.vector.reduce_sum(out=den[:], in_=p16[:], axis=mybir.AxisListType.X)
    nc.vector.reciprocal(out=den[:], in_=den[:])
    pT = singles.tile([P, SC, H], F16)
    t_T16(p16, pT, SC, H, "p")

    # =======================================================================
    # STEP 4: pooled (8, 4096) = pT.T @ llm16, denominator folded in evict;
    # AllToAll (head <-> batch) in f16.
    # =======================================================================
    pool16 = natD.tile([H, D], F16, tag="nat", name="pool16")
    for half in range(2):
        ps_p = psA.tile([H, HD2], F32, tag="big", name=f"ps_pool_{half}")
        for s in range(SC):
            lt = llm_pool.tile([P, HD2], F16, tag="llm", name=f"llm_{half}_{s}")
            nc.sync.dma_start(
                out=lt[:],
                in_=t["llm16"][s * P:(s + 1) * P, half * HD2:(half + 1) * HD2])
            for n in range(HD2 // 512):
                nc.tensor.matmul(ps_p[:, n * 512:(n + 1) * 512],
                                 pT[:, s, :], lt[:, n * 512:(n + 1) * 512],
                                 start=(s == 0), stop=(s == SC - 1))
        nc.vector.tensor_scalar_mul(out=pool16[:, half * HD2:(half + 1) * HD2],
                                    in0=ps_p[:], scalar1=den[:])

    nc.gpsimd.dma_start(out=t["cc_pool_in"][:], in_=pool16[:])
    nc.gpsimd.collective_compute(
        "AllToAll", ALU.bypass, replica_groups=GROUPS,
        ins=[t["cc_pool_in"][:].opt()], outs=[t["cc_pool_out"][:].opt()])

    # =======================================================================
    # STEP 5: ctx for this core's head, all batches: (8, 512) = poolh@wv + bv
    # =======================================================================
    poolh16 = natD.tile([B, D], F16, tag="nat", name="poolh16")
    nc.gpsimd.dma_start(out=poolh16[:], in_=t["cc_pool_out"][:])
    poolhT = singles.tile([P, DC, B], F16)
    t_T16(poolh16, poolhT, DC, B, "ph")

    ps_cx = psC.tile([B, DH], F32, tag="mix", name="ps_cx")
    wv_r = t["wv_s"].rearrange("(c p) n -> p c n", p=P)
    for g in range(8):
        wt = wvo_pool.tile([P, 4, DH], F16, tag="wvo", name=f"wv_g{g}")
        nc.scalar.dma_start(out=wt[:], in_=wv_r[:, 4 * g:4 * g + 4, :])
        for j in range(4):
            k = 4 * g + j
            nc.tensor.matmul(ps_cx[:], poolhT[:, k, :], wt[:, j, :],
                             start=(k == 0), stop=False)
    bias_mm(ps_cx, bv_sb, DH)
    ctx16 = singles.tile([B, DH], F16)
    evict(ctx16[:], ps_cx[:])
    ctxT = singles.tile([P, DH // P, B], F16)
    t_T16(ctx16, ctxT, DH // P, B, "cx")

    # =======================================================================
    # STEP 6: attn partial (8, 4096) = ctx @ wo_s + bo/8 ; AllReduce f16
    # =======================================================================
    attn16 = natD.tile([B, D], F16, tag="nat", name="attn16")
    for half in range(2):
        ps_a = psA.tile([B, HD2], F32, tag="big", name=f"ps_attn_{half}")
        for k in range(DH // P):
            wt = wvo_pool.tile([P, HD2], F16, tag="wvo", name=f"wo_t{half}_{k}")
            nc.scalar.dma_start(
                out=wt[:],
                in_=t["wo_s"][k * P:(k + 1) * P, half * HD2:(half + 1) * HD2])
            for n in range(HD2 // 512):
                nc.tensor.matmul(ps_a[:, n * 512:(n + 1) * 512],
                                 ctxT[:, k, :], wt[:, n * 512:(n + 1) * 512],
                                 start=(k == 0), stop=False)
        bias_mm(ps_a, bo_sb[:, half * HD2:(half + 1) * HD2], HD2)
        evict(attn16[:, half * HD2:(half + 1) * HD2], ps_a[:])
    nc.gpsimd.dma_start(out=t["cc_attn_in"][:], in_=attn16[:])
    nc.gpsimd.collective_compute(
        "AllReduce", ALU.add, replica_groups=GROUPS,
        ins=[t["cc_attn_in"][:].opt()], outs=[t["cc_attn_out"][:].opt()])

    # =======================================================================
    # STEP 7: y = LN(attn_out)*g+b ; mlp partial (+b1, gelu, @w2 + b2/8) ; AR
    # =======================================================================
    attn16p = singles.tile([B, D], F16)  # persists (residual)
    nc.gpsimd.dma_start(out=attn16p[:], in_=t["cc_attn_out"][:])

    y16 = natD.tile([B, D], F16, tag="nat", name="y16")
    layernorm16(attn16p[:], B, D, y16[:], "ln0")
    yT = singles.tile([P, DC, B], F16)
    t_T16(y16, yT, DC, B, "y", g=lng_sb, b=lnb_sb)

    # mm1: h1 (8, 2048) = y @ w1_s + b1 ; exact gelu straight off PSUM
    ps_h1 = psA.tile([B, F1S], F32, tag="big", name="ps_h1")
    for k in range(DC):
        wt = w1_pool.tile([P, F1S], F16, tag="w1", name=f"w1_t{k}")
        nc.scalar.dma_start(out=wt[:], in_=t["w1_s"][k * P:(k + 1) * P, :])
        for n in range(F1S // 512):
            nc.tensor.matmul(ps_h1[:, n * 512:(n + 1) * 512],
                             yT[:, k, :], wt[:, n * 512:(n + 1) * 512],
                             start=(k == 0), stop=False)
    bias_mm(ps_h1, b1_sb, F1S)
    g16 = natD.tile([B, F1S], F16, tag="nat", name="g16")
    nc.scalar.activation(out=g16[:], in_=ps_h1[:], func=AF.Gelu)
    gT = singles.tile([P, F1S // P, B], F16)
    t_T16(g16, gT, F1S // P, B, "g")

    # tail weight packs on the scalar queue, behind w1 (used from STEP 9).
    rp_sb = []
    for j in range(DC // 4):
        wt = w1_pool.tile([P, 4, HID], F16, tag="w1", name=f"rp_{j}")
        nc.scalar.dma_start(out=wt[:], in_=t["rinp"][:, 4 * j:4 * j + 4, :])
        rp_sb.append(wt)
    bw1_sb = []
    for i in range(NBLK):
        for k in range(HC):
            wt = singles.tile([P, 4 * HID], F16, name=f"bw1_{i}_{k}")
            nc.scalar.dma_start(out=wt[:], in_=t["blkw1p"][:, i * HC + k, :])
            bw1_sb.append(wt)
    bw2_sb = []
    for i in range(NBLK):
        wt = singles.tile([P, 8, HID], F16, name=f"bw2_{i}")
        nc.scalar.dma_start(out=wt[:], in_=t["blkw2p"][:, i, :].rearrange(
            "p (a n) -> p a n", a=8))
        bw2_sb.append(wt)

    # mm2: h2 partial (8, 4096) = g @ w2_s + b2/8 ; AllReduce f16
    h216 = natD.tile([B, D], F16, tag="nat", name="h216")
    for half in range(2):
        ps_h2 = psA.tile([B, HD2], F32, tag="big", name=f"ps_h2_{half}")
        for k in range(F1S // P):
            wt = w2_pool.tile([P, HD2], F16, tag="w2", name=f"w2_t{half}_{k}")
            nc.sync.dma_start(
                out=wt[:],
                in_=t["w2_s"][k * P:(k + 1) * P, half * HD2:(half + 1) * HD2])
            for n in range(HD2 // 512):
                nc.tensor.matmul(ps_h2[:, n * 512:(n + 1) * 512],
                                 gT[:, k, :], wt[:, n * 512:(n + 1) * 512],
                                 start=(k == 0), stop=False)
        bias_mm(ps_h2, b2_sb[:, half * HD2:(half + 1) * HD2], HD2)
        evict(h216[:, half * HD2:(half + 1) * HD2], ps_h2[:])
    nc.gpsimd.dma_start(out=t["cc_mlp_in"][:], in_=h216[:])
    nc.gpsimd.collective_compute(
        "AllReduce", ALU.add, replica_groups=GROUPS,
        ins=[t["cc_mlp_in"][:].opt()], outs=[t["cc_mlp_out"][:].opt()])

    # =======================================================================
    # STEP 8: x_pool = attn_out + h ; x0 = x_pool @ rin_pool (+ early parts)
    # =======================================================================
    h216p = natD.tile([B, D], F16, tag="nat", name="h216p")
    nc.gpsimd.dma_start(out=h216p[:], in_=t["cc_mlp_out"][:])
    xp16 = natD.tile([B, D], F16, tag="nat", name="xp16")
    nc.vector.tensor_add(out=xp16[:], in0=attn16p[:], in1=h216p[:])
    xpT = singles.tile([P, DC, B], F16)
    t_T16(xp16, xpT, DC, B, "xp")

    for k in range(DC):
        nc.tensor.matmul(ps_x0[:], xpT[:, k, :], rp_sb[k // 4][:, k % 4, :],
                         start=False, stop=(k == DC - 1))
    x_nat = singles.tile([B, HID], F32)
    evict(x_nat[:], ps_x0[:])

    # ---- 3 residual blocks ----
    for i in range(NBLK):
        xn16 = singles.tile([B, HID], F16, name=f"xn_{i}")
        layernorm16(x_nat[:], B, HID, xn16[:], f"lnb{i}")
        xnT = singles.tile([P, HC, B], F16, name=f"xnT_{i}")
        t_T16(xn16, xnT, HC, B, f"xn{i}",
              g=bgr_sb[:, i, :], b=bbr_sb[:, i, :])

        ps_bh = psA.tile([B, 4 * HID], F32, tag="big", name=f"ps_bh_{i}")
        for k in range(HC):
            for n in range(4 * HID // 512):
                nc.tensor.matmul(ps_bh[:, n * 512:(n + 1) * 512],
                                 xnT[:, k, :],
                                 bw1_sb[i * HC + k][:, n * 512:(n + 1) * 512],
                                 start=(k == 0), stop=False)
        bias_mm(ps_bh, bb1_sb[:, i, :], 4 * HID)
        hb16 = singles.tile([B, 4 * HID], F16, name=f"hb_{i}")
        nc.scalar.activation(out=hb16[:], in_=ps_bh[:], func=AF.Silu)
        hbT = singles.tile([P, 4 * HID // P, B], F16, name=f"hbT_{i}")
        t_T16(hb16, hbT, 4 * HID // P, B, f"hb{i}")

        ps_bo = psC.tile([B, HID], F32, tag="mix", name=f"ps_bo_{i}")
        for k in range(4 * HID // P):
            nc.tensor.matmul(ps_bo[:], hbT[:, k, :], bw2_sb[i][:, k, :],
                             start=(k == 0), stop=False)
        bias_mm(ps_bo, bb2_sb[:, i, :], HID)
        nc.vector.tensor_add(out=x_nat[:], in0=x_nat[:], in1=ps_bo[:])

    # ---- final: res (8, 7) = swish(x) @ out_w + out_b
    xs16 = singles.tile([B, HID], F16)
    nc.scalar.activation(out=xs16[:], in_=x_nat[:], func=AF.Silu)
    xsT = singles.tile([P, HC, B], F16)
    t_T16(xs16, xsT, HC, B, "xs")
    ps_o = psC.tile([B, AD], F32, tag="mix", name="ps_o")
    for k in range(HC):
        nc.tensor.matmul(ps_o[:], xsT[:, k, :], ow_sb[:, k, :],
                         start=(k == 0), stop=(k == HC - 1))
    out_sb = singles.tile([B, AD], F32)
    nc.vector.tensor_add(out=out_sb[:], in0=ps_o[:], in1=ob_bc[:])
    nc.sync.dma_start(out=t["res"][:], in_=out_sb[:])


_CACHED_NC = None


def _get_nc():
    global _CACHED_NC
    if _CACHED_NC is None:
        _CACHED_NC = build_program()
    return _CACHED_NC


def _prep_in_maps(inputs):
    f32 = np.float32
    f16 = np.float16
    f8 = ml_dtypes.float8_e3m4
    llm_full = np.ascontiguousarray(np.asarray(inputs["llm_output"], dtype=f32))
    wv = np.asarray(inputs["wv"], f32); wo = np.asarray(inputs["wo"], f32)
    bv = np.asarray(inputs["bv"], f32); bo = np.asarray(inputs["bo"], f32)
    w1 = np.asarray(inputs["mlp_w1"], f32); b1 = np.asarray(inputs["mlp_b1"], f32)
    w2 = np.asarray(inputs["mlp_w2"], f32); b2 = np.asarray(inputs["mlp_b2"], f32)
    rin_w = np.asarray(inputs["rin_w"], f32)

    # host-folded probe projection: U[:, h] = wk_h @ q_h / sqrt(DH)
    probe = np.asarray(inputs["probe"], np.float64).reshape(D)
    wq = np.asarray(inputs["wq"], np.float64)
    wk = np.asarray(inputs["wk"], np.float64)
    bq = np.asarray(inputs["bq"], np.float64)
    q = probe @ wq + bq
    U = np.zeros((D, H))
    for h in range(H):
        hb = slice(h * DH, (h + 1) * DH)
        U[:, h] = wk[:, hb] @ q[hb] / np.sqrt(DH)
    U16r = np.ascontiguousarray(
        (U / LLM_SCALE).reshape(DC, P, H).transpose(1, 0, 2)).astype(f16)

    def r128(v):  # (n*128,) -> (128, n) partition-major
        return np.ascontiguousarray(v.reshape(-1, P).T)

    blk_g = np.asarray(inputs["blk_ln_g"], f32)
    blk_b = np.asarray(inputs["blk_ln_b"], f32)
    blkw1 = np.asarray(inputs["blk_w1"], f32).astype(f16)    # (N, HID, 4HID)
    blkw2 = np.asarray(inputs["blk_w2"], f32).astype(f16)    # (N, 4HID, HID)
    rinp = np.ascontiguousarray(
        rin_w[TD:TD + D].reshape(DC, P, HID).transpose(1, 0, 2)).astype(f16)

    shared = {
        "U16r": U16r,
        "bo16": (bo / NC).astype(f16).reshape(1, D),
        "ln_g_r": r128(np.asarray(inputs["ln_g"], f32)),
        "ln_b_r": r128(np.asarray(inputs["ln_b"], f32)),
        "b216": (b2 / NC).astype(f16).reshape(1, D),
        "four_w2": np.concatenate(
            [np.asarray(inputs["four_w"], f32).reshape(TD // 2, 1)] * 2),
        "phase2": np.concatenate(
            [np.full((TD // 2, 1), np.pi / 2, f32),
             np.zeros((TD // 2, 1), f32)]),
        "timeT": np.ascontiguousarray(np.asarray(inputs["time"], f32).T),
        "naT": np.ascontiguousarray(
            np.asarray(inputs["noisy_actions"], f32).T).astype(f16),
        "cond_w1": np.asarray(inputs["cond_w1"], f32).astype(f16),
        "cond_b1c": np.asarray(inputs["cond_b1"], f32).reshape(-1, 1),
        "cond_w2": np.asarray(inputs["cond_w2"], f32).astype(f16),
        "cond_b2c": np.asarray(inputs["cond_b2"], f32).reshape(-1, 1),
        "rin_cond": np.ascontiguousarray(rin_w[0:TD]).astype(f16),
        "rinp": rinp,
        "rin_na": np.ascontiguousarray(rin_w[TD + D:]).astype(f16),
        "rb16": np.asarray(inputs["rin_b"], f32).astype(f16).reshape(1, HID),
        "blk_g_r": np.ascontiguousarray(
            blk_g.reshape(NBLK, HC, P).transpose(0, 2, 1)),
        "blk_b_r": np.ascontiguousarray(
            blk_b.reshape(NBLK, HC, P).transpose(0, 2, 1)),
        "blkw1p": np.ascontiguousarray(
            blkw1.reshape(NBLK, HC, P, 4 * HID).transpose(2, 0, 1, 3)
            .reshape(P, NBLK * HC, 4 * HID)),
        "blk_b1_16": np.asarray(inputs["blk_b1"], f32).astype(f16),
        "blkw2p": np.ascontiguousarray(
            blkw2.reshape(NBLK, 8, P, HID).transpose(2, 0, 1, 3)
            .reshape(P, NBLK, 8 * HID)),
        "blk_b2_16": np.asarray(inputs["blk_b2"], f32).astype(f16),
        "out_w": np.ascontiguousarray(
            np.asarray(inputs["out_w"], f32).astype(f16)
            .reshape(HC, P, AD).transpose(1, 0, 2)),
        "out_bc": np.asarray(inputs["out_b"], f32).reshape(1, AD),
    }

    in_maps = []
    for i in range(NC):
        hb = slice(i * DH, (i + 1) * DH)
        fb = slice(i * F1S, (i + 1) * F1S)
        m = dict(shared)
        m["llm16"] = llm_full[i].astype(f16)
        m["llmT8"] = np.clip(
            llm_full[i].T * LLM_SCALE, -15.5, 15.5).astype(f8)
        m["wv_s"] = np.ascontiguousarray(wv[:, hb]).astype(f16)
        m["bv16"] = np.ascontiguousarray(bv[hb]).astype(f16).reshape(1, DH)
        m["wo_s"] = np.ascontiguousarray(wo[hb, :]).astype(f16)
        m["w1_s"] = np.ascontiguousarray(w1[:, fb]).astype(f16)
        m["b116"] = np.ascontiguousarray(b1[fb]).astype(f16).reshape(1, F1S)
        m["w2_s"] = np.ascontiguousarray(w2[fb, :]).astype(f16)
        in_maps.append(m)
    return in_maps


def kernel(**inputs):
    nc = _get_nc()
    in_maps = _prep_in_maps(inputs)
    r = run_bass_kernel_spmd(nc, in_maps, core_ids=list(range(NC)))
    return np.ascontiguousarray(r.results[0]["res"]).astype(np.float32)


def run_traced(**inputs):
    """Like kernel() but with NTFF tracing; returns (output, results)."""
    nc = _get_nc()
    in_maps = _prep_in_maps(inputs)
    r = run_bass_kernel_spmd(nc, in_maps, core_ids=list(range(NC)), trace=True)
    return np.ascontiguousarray(r.results[0]["res"]).astype(np.float32), r
